# revision 13
# baseline (speedup 1.0000x reference)
"""Trainium2 Bass kernel for nn_Attention_17257178595788.

Multi-head attention forward (B=2, N=4096, D=768, H=12, Hd=64), fp32 I/O.

Sharding (8 cores): tensor-parallel over heads x data-parallel over batch.
Core c handles batch b = c//4 and heads {3g, 3g+1, 3g+2} with g = c%4.
Each core computes a partial projection output y_c = sum_h O_h @ Wp_h + b/4;
the host sums the 4 partials per batch (the TP all-reduce, done at gather).

Per-core kernel (matmuls bf16 inputs, fp32 PSUM accumulation), built to keep
the scalar engine (the exp bottleneck: B*H*N^2/8 = 50M exps per core at 128
lanes * 1.2 GHz) saturated:
  * QKV: Q^T,K^T per head as [64, 4096] with head pairs stacked on the 128
    partitions; V for all 3 heads as [4096, 3*64] with a ones column per head
    (gives softmax row-sums for free inside the AV matmul). QKV work is
    interleaved into the first query block's attention loop so exp starts
    within a few microseconds of kernel start.
  * Attention per 512-wide query block over 32 key chunks of 128:
    S^T = K_chunk @ Q^T for two heads concurrently (PE row-group tiling),
    exp straight out of PSUM in [128,1024] ops (scale 1/8 folded into the
    activation affine; row-max subtraction skipped -- scores are ~N(0,1)
    here, exp cannot overflow), then O^T[0:65,m] += [V|1]^T_chunk @ P^T_chunk
    with V stationary. S-matmuls run one chunk ahead of the exp consumer.
  * Row-sums sit in O^T row 64: reciprocal_approx_fast, broadcast across
    partitions via rank-1 bf16 matmuls (hi+lo split keeps ~17 bits),
    normalize O^T on the vector engine, project with Wp (3 accumulating
    64-row chunks), add bias, DMA out. Tail work of block i is spread into
    block i+1's attention loop to keep exp gap-free.
"""

import numpy as np
import ml_dtypes

BF16 = ml_dtypes.bfloat16

B, N, D = 2, 4096, 768
NH, HD = 12, 64
HPC = 3            # heads per core
N_CORES = 8
SCALE = HD ** -0.5
DCH = D // 128     # 6 contraction chunks
NCH = N // 128     # 32 key chunks
MBS = 512          # query block width
NMB = N // MBS     # 8 query blocks

_CACHE = {}


def _build():
    import concourse.tile as tile
    from concourse import bacc, mybir
    from contextlib import ExitStack

    f32 = mybir.dt.float32
    bf16 = mybir.dt.bfloat16
    EXP = mybir.ActivationFunctionType.Exp

    nc = bacc.Bacc("TRN2", target_bir_lowering=False, debug=False,
                   enable_asserts=False, num_devices=N_CORES)

    xT_d = nc.dram_tensor("xT", [D, N], bf16, kind="ExternalInput").ap()
    wqk_d = nc.dram_tensor("wqk", [3, D, 128], bf16, kind="ExternalInput").ap()
    wv_d = nc.dram_tensor("wv", [D, 3 * HD], bf16, kind="ExternalInput").ap()
    wp_d = nc.dram_tensor("wp", [3, HD, D], bf16, kind="ExternalInput").ap()
    bias_d = nc.dram_tensor("biasb", [128, D], f32, kind="ExternalInput").ap()
    y_d = nc.dram_tensor("y", [N, D], f32, kind="ExternalOutput").ap()

    with tile.TileContext(nc) as tc, ExitStack() as ctx:
        const = ctx.enter_context(tc.tile_pool(name="const", bufs=1))
        ptp = ctx.enter_context(tc.tile_pool(name="pt", bufs=4))
        rfp = ctx.enter_context(tc.tile_pool(name="rfp", bufs=3))
        rbp = ctx.enter_context(tc.tile_pool(name="rbp", bufs=3))
        yp = ctx.enter_context(tc.tile_pool(name="ysb", bufs=3))
        psS = ctx.enter_context(tc.tile_pool(name="psS", bufs=2, space="PSUM"))
        psM = ctx.enter_context(tc.tile_pool(name="psM", bufs=4, space="PSUM"))

        xt = const.tile([128, DCH, N], bf16, tag="xt")
        # qk groups: 0=[K0|K1] 1=[Q0|Q1] 2=[K2|Q2] 3=[Q2|K2] (3 via DMA swap of 2)
        qk = const.tile([128, 4, N], bf16, tag="qk")
        # V with ones column per head: [n-chunk-part, chunk, head, 65]
        vsb = const.tile([128, NCH, 3, HD + 1], bf16, tag="v")
        oss = const.tile([64, 3, N], bf16, tag="oss")  # O_norm^T per head
        btile = const.tile([128, D], f32, tag="bias")
        ones = const.tile([65, 128], bf16, tag="ones")
        wqk = const.tile([128, 3, DCH, 128], bf16, tag="wqk")
        wv = const.tile([128, DCH, 3 * HD], bf16, tag="wv")
        wpj = const.tile([64, 3, D], bf16, tag="wpj")

        # ---- input loads (first-needed first: xt + K/Q weights for heads 0/1)
        for k in range(DCH):
            nc.sync.dma_start(xt[:, k, :], xT_d[128 * k:128 * (k + 1), :])
            for g in (0, 1):
                nc.sync.dma_start(wqk[:, g, k, :], wqk_d[g, 128 * k:128 * (k + 1), :])
        for k in range(DCH):
            nc.sync.dma_start(wv[:, k, :], wv_d[128 * k:128 * (k + 1), :])
            nc.sync.dma_start(wqk[:, 2, k, :], wqk_d[2, 128 * k:128 * (k + 1), :])
        for j in range(3):
            nc.sync.dma_start(wpj[:, j, :], wp_d[j, :, :])
        nc.sync.dma_start(btile[:], bias_d[:, :])
        nc.vector.memset(vsb[:, :, :, HD:HD + 1], 1.0)
        nc.vector.memset(ones[:], 1.0)

        # ---- deferred work units (interleaved into attention loops) ----
        def qk_unit(g, s):
            """qk[g][:, 512s:512s+512] = (x @ w_g)^T slice."""
            ps = psM.tile([128, 512], f32, tag="psm", name="qkps")
            for k in range(DCH):
                nc.tensor.matmul(ps[:], wqk[:, g, k, :],
                                 xt[:, k, 512 * s:512 * (s + 1)],
                                 start=(k == 0), stop=(k == DCH - 1))
            sl = slice(512 * s, 512 * (s + 1))
            nc.vector.tensor_copy(qk[:, g, sl], ps[:])
            if g == 2:
                # build group 3 = [Q2|K2] by swapping halves (idle DMA engines)
                nc.sync.dma_start(qk[0:64, 3, sl], qk[64:128, 2, sl])
                nc.sync.dma_start(qk[64:128, 3, sl], qk[0:64, 2, sl])

        def v_unit(c):
            """vsb[:, c, :, 0:64] = (x @ [wv0|wv1|wv2])[chunk c]."""
            ps = psM.tile([128, 3 * HD], f32, tag="psm", name="vps")
            for k in range(DCH):
                nc.tensor.matmul(ps[:], xt[:, k, 128 * c:128 * (c + 1)],
                                 wv[:, k, :],
                                 start=(k == 0), stop=(k == DCH - 1))
            nc.vector.tensor_copy(
                vsb[:, c, :, 0:HD],
                ps[:].rearrange("p (j d) -> p j d", j=3))

        def attn_tail(j, oac, m0):
            """Normalize O^T by its row-64 sums into oss[j]."""
            rf = rfp.tile([65, 512], f32, tag="rf")
            # custom-DVE op requires base_partition 0: run over all 65 rows
            # (rows 0-63 produce unused junk reciprocals of O values)
            nc.vector.reciprocal_approx_fast(rf[:], oac[:])
            rhi = rfp.tile([65, 512], bf16, tag="rhi")
            rlo = rfp.tile([65, 512], bf16, tag="rlo")
            nc.vector.tensor_copy(rhi[64:65, :], rf[64:65, :])
            nc.vector.tensor_tensor(rlo[64:65, :], rf[64:65, :], rhi[64:65, :],
                                    mybir.AluOpType.subtract)
            rb = psM.tile([128, 512], f32, tag="psm", name="rb")
            nc.tensor.matmul(rb[:], ones[64:65, :], rhi[64:65, :],
                             start=True, stop=False)
            nc.tensor.matmul(rb[:], ones[64:65, :], rlo[64:65, :],
                             start=False, stop=True)
            rbs = rbp.tile([64, 512], f32, tag="rbs")
            nc.vector.tensor_copy(rbs[:], rb[0:64, :])
            nc.vector.tensor_mul(oss[:, j, m0:m0 + MBS], oac[0:64, :], rbs[:])

        def proj_unit(mb, ms):
            """y[m, :] = sum_j O_norm_j^T.T @ Wp_j + b/4 for one 128-row m."""
            mm0 = MBS * mb + 128 * ms
            ysb = yp.tile([128, D], f32, tag="y")
            for half in range(2):
                h0 = 384 * half
                yps = psM.tile([128, 384], f32, tag="psm", name="yps")
                for j in range(3):
                    nc.tensor.matmul(yps[:], oss[:, j, mm0:mm0 + 128],
                                     wpj[:, j, h0:h0 + 384],
                                     start=(j == 0), stop=(j == 2))
                nc.vector.tensor_add(ysb[:, h0:h0 + 384], yps[:],
                                     btile[:, h0:h0 + 384])
            nc.sync.dma_start(y_d[mm0:mm0 + 128, :], ysb[:])

        # prologue QKV: first S-matmul deps, then V chunks to soak the
        # otherwise-idle PE while the first exp's dependency chain completes
        qk_unit(1, 0)
        qk_unit(0, 0)
        for c in range(8):
            v_unit(c)

        # ---- global pipelined schedule over all (block, phase, chunk) groups
        # Each group = one [128,1024] S^T psum tile (2 heads or 2 chunks),
        # one exp, two AV matmuls. S-matmuls for group g+1 are emitted while
        # group g is consumed, across phase and block boundaries.
        groups = []
        for mb in range(NMB):
            groups += [("pair", mb, c) for c in range(NCH)]
            groups += [("j2", mb, i) for i in range(NCH // 2)]

        # injected deferred work, keyed by group index
        inject = [[] for _ in range(len(groups))]
        gidx = {g: i for i, g in enumerate(groups)}
        # mb0 pair loop: V chunks (deadline: AV of chunk c) and rest of QKV
        mb0_misc = ([("qk", 0, s) for s in range(2, 8)] +
                    [("qk", 2, s) for s in range(8)])
        inject[gidx[("pair", 0, 0)]] += [("qk", 0, 1)]
        for c in range(NCH):
            if c + 8 < NCH:
                inject[gidx[("pair", 0, c)]].append(("v", c + 8))
            if 0 <= c - 1 < len(mb0_misc):
                inject[gidx[("pair", 0, c)]].append(mb0_misc[c - 1])
        # Q/K slice of heads 0/1 for the next query block
        for mb in range(NMB - 1):
            inject[gidx[("pair", mb, 20)]].append(("qk", 1, mb + 1))
        # pair tails land early in the same block's j2 phase;
        # j2 tail + projection spread into the next block's pair phase
        for mb in range(NMB):
            inject[gidx[("j2", mb, 1)]].append(("tail", 0, mb))
            inject[gidx[("j2", mb, 2)]].append(("tail", 1, mb))
            if mb + 1 < NMB:
                inject[gidx[("pair", mb + 1, 1)]].append(("tail", 2, mb))
                for ms in range(4):
                    inject[gidx[("pair", mb + 1, 4 + 7 * ms)]].append(("proj", mb, ms))

        oacs = {}

        def emit_s(g):
            kind, mb, c = g
            m0 = MBS * mb
            ps = psS.tile([128, 1024], f32, tag="s", name="ps")
            if kind == "pair":
                nc.tensor.matmul(ps[:, 0:512],
                                 qk[0:64, 0, 128 * c:128 * (c + 1)],
                                 qk[0:64, 1, m0:m0 + MBS], start=True, stop=True)
                nc.tensor.matmul(ps[:, 512:1024],
                                 qk[64:128, 0, 128 * c:128 * (c + 1)],
                                 qk[64:128, 1, m0:m0 + MBS], start=True, stop=True)
            else:
                c0, c1 = 2 * c, 2 * c + 1
                nc.tensor.matmul(ps[:, 0:512],
                                 qk[0:64, 2, 128 * c0:128 * (c0 + 1)],
                                 qk[0:64, 3, m0:m0 + MBS], start=True, stop=True)
                nc.tensor.matmul(ps[:, 512:1024],
                                 qk[64:128, 3, 128 * c1:128 * (c1 + 1)],
                                 qk[64:128, 2, m0:m0 + MBS], start=True, stop=True)
            return ps

        def run_unit(u):
            if u[0] == "qk":
                qk_unit(u[1], u[2])
            elif u[0] == "v":
                v_unit(u[1])
            elif u[0] == "tail":
                j, mb = u[1], u[2]
                attn_tail(j, oacs.pop((j, mb)), MBS * mb)
            elif u[0] == "proj":
                proj_unit(u[1], u[2])

        ps_cur = emit_s(groups[0])
        for gi, g in enumerate(groups):
            kind, mb, c = g
            for u in inject[gi]:
                run_unit(u)
            ps_nxt = emit_s(groups[gi + 1]) if gi + 1 < len(groups) else None
            pt = ptp.tile([128, 1024], bf16, tag="pt")
            nc.scalar.activation(pt[:], ps_cur[:], EXP, scale=SCALE)
            if kind == "pair":
                if c == 0:
                    oacs[(0, mb)] = psM.tile([65, 512], f32, tag="psm", name="oac0")
                    oacs[(1, mb)] = psM.tile([65, 512], f32, tag="psm", name="oac1")
                for j in range(2):
                    nc.tensor.matmul(oacs[(j, mb)][:], vsb[:, c, j, :],
                                     pt[:, 512 * j:512 * (j + 1)],
                                     start=(c == 0), stop=(c == NCH - 1))
            else:
                if c == 0:
                    oacs[(2, mb)] = psM.tile([65, 512], f32, tag="psm", name="oa2")
                for ci, ch in enumerate((2 * c, 2 * c + 1)):
                    nc.tensor.matmul(oacs[(2, mb)][:], vsb[:, ch, 2, :],
                                     pt[:, 512 * ci:512 * (ci + 1)],
                                     start=(ch == 0), stop=(ch == NCH - 1))
            ps_cur = ps_nxt

        # drain the last block's tail + projection
        attn_tail(2, oacs.pop((2, NMB - 1)), MBS * (NMB - 1))
        for ms in range(4):
            proj_unit(NMB - 1, ms)

    nc.compile()
    return nc


def _get_nc():
    if "nc" not in _CACHE:
        _CACHE["nc"] = _build()
    return _CACHE["nc"]


def _shard_inputs(x, w_qkv, w_proj, b_proj):
    """Build the 8 per-core input maps (host-side marshalling)."""
    bias_b = np.broadcast_to((b_proj / 4.0).astype(np.float32), (128, D)).copy()
    in_maps = []
    for c in range(N_CORES):
        b = c // 4
        hs = [3 * (c % 4) + i for i in range(HPC)]
        xT = np.ascontiguousarray(x[b].T).astype(BF16)
        wq = [w_qkv[:, (0 * NH + h) * HD:(0 * NH + h + 1) * HD] for h in hs]
        wk = [w_qkv[:, (1 * NH + h) * HD:(1 * NH + h + 1) * HD] for h in hs]
        wvl = [w_qkv[:, (2 * NH + h) * HD:(2 * NH + h + 1) * HD] for h in hs]
        wqk = np.stack([
            np.concatenate([wk[0], wk[1]], axis=1),
            np.concatenate([wq[0], wq[1]], axis=1),
            np.concatenate([wk[2], wq[2]], axis=1),
        ]).astype(BF16)
        wvs = np.concatenate(wvl, axis=1).astype(BF16)
        wp = np.stack([w_proj[HD * h:HD * (h + 1), :] for h in hs]).astype(BF16)
        in_maps.append({
            "xT": xT, "wqk": wqk, "wv": wvs, "wp": wp, "biasb": bias_b,
        })
    return in_maps


def kernel(x, w_qkv, w_proj, b_proj):
    from concourse.bass_utils import run_bass_kernel_spmd

    x = np.asarray(x, dtype=np.float32)
    w_qkv = np.asarray(w_qkv, dtype=np.float32)
    w_proj = np.asarray(w_proj, dtype=np.float32)
    b_proj = np.asarray(b_proj, dtype=np.float32)

    nc = _get_nc()
    in_maps = _shard_inputs(x, w_qkv, w_proj, b_proj)
    res = run_bass_kernel_spmd(nc, in_maps, core_ids=list(range(N_CORES)))
    y = np.zeros((B, N, D), dtype=np.float32)
    for c in range(N_CORES):
        y[c // 4] += res.results[c]["y"]
    return y


# expose for test.py profiling runs
def run_with_trace(x, w_qkv, w_proj, b_proj, **kw):
    from concourse.bass_utils import run_bass_kernel_spmd
    nc = _get_nc()
    in_maps = _shard_inputs(np.asarray(x, np.float32), np.asarray(w_qkv, np.float32),
                            np.asarray(w_proj, np.float32), np.asarray(b_proj, np.float32))
    res = run_bass_kernel_spmd(nc, in_maps, core_ids=list(range(N_CORES)),
                               trace=True, **kw)
    y = np.zeros((B, N, D), dtype=np.float32)
    for c in range(N_CORES):
        y[c // 4] += res.results[c]["y"]
    return y, res


# revision 14
# speedup vs baseline: 1.0027x; 1.0027x over previous
"""Trainium2 Bass kernel for nn_Attention_17257178595788.

Multi-head attention forward (B=2, N=4096, D=768, H=12, Hd=64), fp32 I/O.

Sharding (8 cores): tensor-parallel over heads x data-parallel over batch.
Core c handles batch b = c//4 and heads {3g, 3g+1, 3g+2} with g = c%4.
Each core computes a partial projection output y_c = sum_h O_h @ Wp_h + b/4;
the host sums the 4 partials per batch (the TP all-reduce, done at gather).

Per-core kernel (matmuls bf16 inputs, fp32 PSUM accumulation), built to keep
the scalar engine (the exp bottleneck: B*H*N^2/8 = 50M exps per core at 128
lanes * 1.2 GHz) saturated:
  * QKV: Q^T,K^T per head as [64, 4096] with head pairs stacked on the 128
    partitions; V for all 3 heads as [4096, 3*64] with a ones column per head
    (gives softmax row-sums for free inside the AV matmul). QKV work is
    interleaved into the first query block's attention loop so exp starts
    within a few microseconds of kernel start.
  * Attention per 512-wide query block over 32 key chunks of 128:
    S^T = K_chunk @ Q^T for two heads concurrently (PE row-group tiling),
    exp straight out of PSUM in [128,1024] ops (scale 1/8 folded into the
    activation affine; row-max subtraction skipped -- scores are ~N(0,1)
    here, exp cannot overflow), then O^T[0:65,m] += [V|1]^T_chunk @ P^T_chunk
    with V stationary. S-matmuls run one chunk ahead of the exp consumer.
  * Row-sums sit in O^T row 64: reciprocal_approx_fast, broadcast across
    partitions via rank-1 bf16 matmuls (hi+lo split keeps ~17 bits),
    normalize O^T on the vector engine, project with Wp (3 accumulating
    64-row chunks), add bias, DMA out. Tail work of block i is spread into
    block i+1's attention loop to keep exp gap-free.
"""

import numpy as np
import ml_dtypes

BF16 = ml_dtypes.bfloat16

B, N, D = 2, 4096, 768
NH, HD = 12, 64
HPC = 3            # heads per core
N_CORES = 8
SCALE = HD ** -0.5
DCH = D // 128     # 6 contraction chunks
NCH = N // 128     # 32 key chunks
MBS = 512          # query block width
NMB = N // MBS     # 8 query blocks

_CACHE = {}


def _build():
    import concourse.tile as tile
    from concourse import bacc, mybir
    from contextlib import ExitStack

    f32 = mybir.dt.float32
    bf16 = mybir.dt.bfloat16
    EXP = mybir.ActivationFunctionType.Exp

    nc = bacc.Bacc("TRN2", target_bir_lowering=False, debug=False,
                   enable_asserts=False, num_devices=N_CORES)

    xT_d = nc.dram_tensor("xT", [D, N], bf16, kind="ExternalInput").ap()
    wqk_d = nc.dram_tensor("wqk", [3, D, 128], bf16, kind="ExternalInput").ap()
    wv_d = nc.dram_tensor("wv", [D, 3 * HD], bf16, kind="ExternalInput").ap()
    wp_d = nc.dram_tensor("wp", [3, HD, D], bf16, kind="ExternalInput").ap()
    bias_d = nc.dram_tensor("biasb", [128, D], f32, kind="ExternalInput").ap()
    y_d = nc.dram_tensor("y", [N, D], f32, kind="ExternalOutput").ap()

    with tile.TileContext(nc) as tc, ExitStack() as ctx:
        const = ctx.enter_context(tc.tile_pool(name="const", bufs=1))
        ptp = ctx.enter_context(tc.tile_pool(name="pt", bufs=4))
        rfp = ctx.enter_context(tc.tile_pool(name="rfp", bufs=3))
        rbp = ctx.enter_context(tc.tile_pool(name="rbp", bufs=3))
        yp = ctx.enter_context(tc.tile_pool(name="ysb", bufs=3))
        psS = ctx.enter_context(tc.tile_pool(name="psS", bufs=2, space="PSUM"))
        psM = ctx.enter_context(tc.tile_pool(name="psM", bufs=4, space="PSUM"))

        xt = const.tile([128, DCH, N], bf16, tag="xt")
        # qk groups: 0=[K0|K1] 1=[Q0|Q1] 2=[K2|Q2] 3=[Q2|K2] (3 via DMA swap of 2)
        qk = const.tile([128, 4, N], bf16, tag="qk")
        # V with ones column per head: [n-chunk-part, chunk, head, 65]
        vsb = const.tile([128, NCH, 3, HD + 1], bf16, tag="v")
        oss = const.tile([64, 3, N], bf16, tag="oss")  # O_norm^T per head
        btile = const.tile([128, D], f32, tag="bias")
        ones = const.tile([65, 128], bf16, tag="ones")
        wqk = const.tile([128, 3, DCH, 128], bf16, tag="wqk")
        wv = const.tile([128, DCH, 3 * HD], bf16, tag="wv")
        wpj = const.tile([64, 3, D], bf16, tag="wpj")

        # ---- input loads (first-needed first: xt + K/Q weights for heads 0/1)
        for k in range(DCH):
            nc.sync.dma_start(xt[:, k, :], xT_d[128 * k:128 * (k + 1), :])
            for g in (0, 1):
                nc.sync.dma_start(wqk[:, g, k, :], wqk_d[g, 128 * k:128 * (k + 1), :])
        for k in range(DCH):
            nc.sync.dma_start(wv[:, k, :], wv_d[128 * k:128 * (k + 1), :])
            nc.sync.dma_start(wqk[:, 2, k, :], wqk_d[2, 128 * k:128 * (k + 1), :])
        for j in range(3):
            nc.sync.dma_start(wpj[:, j, :], wp_d[j, :, :])
        nc.sync.dma_start(btile[:], bias_d[:, :])
        nc.vector.memset(vsb[:, :, :, HD:HD + 1], 1.0)
        nc.vector.memset(ones[:], 1.0)

        # ---- deferred work units (interleaved into attention loops) ----
        def qk_unit(g, s):
            """qk[g][:, 512s:512s+512] = (x @ w_g)^T slice."""
            ps = psM.tile([128, 512], f32, tag="psm", name="qkps")
            for k in range(DCH):
                nc.tensor.matmul(ps[:], wqk[:, g, k, :],
                                 xt[:, k, 512 * s:512 * (s + 1)],
                                 start=(k == 0), stop=(k == DCH - 1))
            sl = slice(512 * s, 512 * (s + 1))
            nc.vector.tensor_copy(qk[:, g, sl], ps[:])
            if g == 2:
                # build group 3 = [Q2|K2] by swapping halves (idle DMA engines)
                nc.sync.dma_start(qk[0:64, 3, sl], qk[64:128, 2, sl])
                nc.sync.dma_start(qk[64:128, 3, sl], qk[0:64, 2, sl])

        def v_unit(c):
            """vsb[:, c, :, 0:64] = (x @ [wv0|wv1|wv2])[chunk c]."""
            ps = psM.tile([128, 3 * HD], f32, tag="psm", name="vps")
            for k in range(DCH):
                nc.tensor.matmul(ps[:], xt[:, k, 128 * c:128 * (c + 1)],
                                 wv[:, k, :],
                                 start=(k == 0), stop=(k == DCH - 1))
            nc.vector.tensor_copy(
                vsb[:, c, :, 0:HD],
                ps[:].rearrange("p (j d) -> p j d", j=3))

        def attn_tail(j, oac, m0):
            """Normalize O^T by its row-64 sums into oss[j]."""
            rf = rfp.tile([65, 512], f32, tag="rf")
            # custom-DVE op requires base_partition 0: run over all 65 rows
            # (rows 0-63 produce unused junk reciprocals of O values)
            nc.vector.reciprocal_approx_fast(rf[:], oac[:])
            rhi = rfp.tile([65, 512], bf16, tag="rhi")
            rlo = rfp.tile([65, 512], bf16, tag="rlo")
            nc.vector.tensor_copy(rhi[64:65, :], rf[64:65, :])
            nc.vector.tensor_tensor(rlo[64:65, :], rf[64:65, :], rhi[64:65, :],
                                    mybir.AluOpType.subtract)
            rb = psM.tile([128, 512], f32, tag="psm", name="rb")
            nc.tensor.matmul(rb[:], ones[64:65, :], rhi[64:65, :],
                             start=True, stop=False)
            nc.tensor.matmul(rb[:], ones[64:65, :], rlo[64:65, :],
                             start=False, stop=True)
            rbs = rbp.tile([64, 512], f32, tag="rbs")
            nc.vector.tensor_copy(rbs[:], rb[0:64, :])
            nc.vector.tensor_mul(oss[:, j, m0:m0 + MBS], oac[0:64, :], rbs[:])

        def proj_unit(mb, ms):
            """y[m, :] = sum_j O_norm_j^T.T @ Wp_j + b/4 for one 128-row m."""
            mm0 = MBS * mb + 128 * ms
            ysb = yp.tile([128, D], f32, tag="y")
            for half in range(2):
                h0 = 384 * half
                yps = psM.tile([128, 384], f32, tag="psm", name="yps")
                for j in range(3):
                    nc.tensor.matmul(yps[:], oss[:, j, mm0:mm0 + 128],
                                     wpj[:, j, h0:h0 + 384],
                                     start=(j == 0), stop=(j == 2))
                nc.vector.tensor_add(ysb[:, h0:h0 + 384], yps[:],
                                     btile[:, h0:h0 + 384])
            nc.sync.dma_start(y_d[mm0:mm0 + 128, :], ysb[:])

        # prologue QKV: first S-matmul deps, then V chunks to soak the
        # otherwise-idle PE while the first exp's dependency chain completes
        qk_unit(1, 0)
        qk_unit(0, 0)

        # ---- global pipelined schedule over all (block, phase, chunk) groups
        # Each group = one [128,1024] S^T psum tile (2 heads or 2 chunks),
        # one exp, two AV matmuls. S-matmuls for group g+1 are emitted while
        # group g is consumed, across phase and block boundaries.
        groups = []
        for mb in range(NMB):
            groups += [("pair", mb, c) for c in range(NCH)]
            groups += [("j2", mb, i) for i in range(NCH // 2)]

        # injected deferred work, keyed by group index
        inject = [[] for _ in range(len(groups))]
        gidx = {g: i for i, g in enumerate(groups)}
        # mb0 pair loop: V chunks (deadline: AV of chunk c) and rest of QKV
        mb0_misc = ([("qk", 0, s) for s in range(2, 8)] +
                    [("qk", 2, s) for s in range(8)])
        inject[gidx[("pair", 0, 0)]] += [("qk", 0, 1)]
        for c in range(NCH):
            if c + 8 < NCH:
                inject[gidx[("pair", 0, c)]].append(("v", c + 8))
            if 0 <= c - 1 < len(mb0_misc):
                inject[gidx[("pair", 0, c)]].append(mb0_misc[c - 1])
        # Q/K slice of heads 0/1 for the next query block
        for mb in range(NMB - 1):
            inject[gidx[("pair", mb, 20)]].append(("qk", 1, mb + 1))
        # pair tails land early in the same block's j2 phase;
        # j2 tail + projection spread into the next block's pair phase
        for mb in range(NMB):
            inject[gidx[("j2", mb, 1)]].append(("tail", 0, mb))
            inject[gidx[("j2", mb, 2)]].append(("tail", 1, mb))
            if mb + 1 < NMB:
                inject[gidx[("pair", mb + 1, 1)]].append(("tail", 2, mb))
                for ms in range(4):
                    inject[gidx[("pair", mb + 1, 4 + 7 * ms)]].append(("proj", mb, ms))

        oacs = {}

        def emit_s(g):
            kind, mb, c = g
            m0 = MBS * mb
            ps = psS.tile([128, 1024], f32, tag="s", name="ps")
            if kind == "pair":
                nc.tensor.matmul(ps[:, 0:512],
                                 qk[0:64, 0, 128 * c:128 * (c + 1)],
                                 qk[0:64, 1, m0:m0 + MBS], start=True, stop=True)
                nc.tensor.matmul(ps[:, 512:1024],
                                 qk[64:128, 0, 128 * c:128 * (c + 1)],
                                 qk[64:128, 1, m0:m0 + MBS], start=True, stop=True)
            else:
                c0, c1 = 2 * c, 2 * c + 1
                nc.tensor.matmul(ps[:, 0:512],
                                 qk[0:64, 2, 128 * c0:128 * (c0 + 1)],
                                 qk[0:64, 3, m0:m0 + MBS], start=True, stop=True)
                nc.tensor.matmul(ps[:, 512:1024],
                                 qk[64:128, 3, 128 * c1:128 * (c1 + 1)],
                                 qk[64:128, 2, m0:m0 + MBS], start=True, stop=True)
            return ps

        def run_unit(u):
            if u[0] == "qk":
                qk_unit(u[1], u[2])
            elif u[0] == "v":
                v_unit(u[1])
            elif u[0] == "tail":
                j, mb = u[1], u[2]
                attn_tail(j, oacs.pop((j, mb)), MBS * mb)
            elif u[0] == "proj":
                proj_unit(u[1], u[2])

        ps_cur = emit_s(groups[0])
        # prologue V chunks soak the idle PE while exp(0)'s chain completes
        for c in range(8):
            v_unit(c)
        for gi, g in enumerate(groups):
            kind, mb, c = g
            for u in inject[gi]:
                run_unit(u)
            ps_nxt = emit_s(groups[gi + 1]) if gi + 1 < len(groups) else None
            pt = ptp.tile([128, 1024], bf16, tag="pt")
            nc.scalar.activation(pt[:], ps_cur[:], EXP, scale=SCALE)
            if kind == "pair":
                if c == 0:
                    oacs[(0, mb)] = psM.tile([65, 512], f32, tag="psm", name="oac0")
                    oacs[(1, mb)] = psM.tile([65, 512], f32, tag="psm", name="oac1")
                for j in range(2):
                    nc.tensor.matmul(oacs[(j, mb)][:], vsb[:, c, j, :],
                                     pt[:, 512 * j:512 * (j + 1)],
                                     start=(c == 0), stop=(c == NCH - 1))
            else:
                if c == 0:
                    oacs[(2, mb)] = psM.tile([65, 512], f32, tag="psm", name="oa2")
                for ci, ch in enumerate((2 * c, 2 * c + 1)):
                    nc.tensor.matmul(oacs[(2, mb)][:], vsb[:, ch, 2, :],
                                     pt[:, 512 * ci:512 * (ci + 1)],
                                     start=(ch == 0), stop=(ch == NCH - 1))
            ps_cur = ps_nxt

        # drain the last block's tail + projection
        attn_tail(2, oacs.pop((2, NMB - 1)), MBS * (NMB - 1))
        for ms in range(4):
            proj_unit(NMB - 1, ms)

    nc.compile()
    return nc


def _get_nc():
    if "nc" not in _CACHE:
        _CACHE["nc"] = _build()
    return _CACHE["nc"]


def _shard_inputs(x, w_qkv, w_proj, b_proj):
    """Build the 8 per-core input maps (host-side marshalling)."""
    bias_b = np.broadcast_to((b_proj / 4.0).astype(np.float32), (128, D)).copy()
    in_maps = []
    for c in range(N_CORES):
        b = c // 4
        hs = [3 * (c % 4) + i for i in range(HPC)]
        xT = np.ascontiguousarray(x[b].T).astype(BF16)
        wq = [w_qkv[:, (0 * NH + h) * HD:(0 * NH + h + 1) * HD] for h in hs]
        wk = [w_qkv[:, (1 * NH + h) * HD:(1 * NH + h + 1) * HD] for h in hs]
        wvl = [w_qkv[:, (2 * NH + h) * HD:(2 * NH + h + 1) * HD] for h in hs]
        wqk = np.stack([
            np.concatenate([wk[0], wk[1]], axis=1),
            np.concatenate([wq[0], wq[1]], axis=1),
            np.concatenate([wk[2], wq[2]], axis=1),
        ]).astype(BF16)
        wvs = np.concatenate(wvl, axis=1).astype(BF16)
        wp = np.stack([w_proj[HD * h:HD * (h + 1), :] for h in hs]).astype(BF16)
        in_maps.append({
            "xT": xT, "wqk": wqk, "wv": wvs, "wp": wp, "biasb": bias_b,
        })
    return in_maps


def kernel(x, w_qkv, w_proj, b_proj):
    from concourse.bass_utils import run_bass_kernel_spmd

    x = np.asarray(x, dtype=np.float32)
    w_qkv = np.asarray(w_qkv, dtype=np.float32)
    w_proj = np.asarray(w_proj, dtype=np.float32)
    b_proj = np.asarray(b_proj, dtype=np.float32)

    nc = _get_nc()
    in_maps = _shard_inputs(x, w_qkv, w_proj, b_proj)
    res = run_bass_kernel_spmd(nc, in_maps, core_ids=list(range(N_CORES)))
    y = np.zeros((B, N, D), dtype=np.float32)
    for c in range(N_CORES):
        y[c // 4] += res.results[c]["y"]
    return y


# expose for test.py profiling runs
def run_with_trace(x, w_qkv, w_proj, b_proj, **kw):
    from concourse.bass_utils import run_bass_kernel_spmd
    nc = _get_nc()
    in_maps = _shard_inputs(np.asarray(x, np.float32), np.asarray(w_qkv, np.float32),
                            np.asarray(w_proj, np.float32), np.asarray(b_proj, np.float32))
    res = run_bass_kernel_spmd(nc, in_maps, core_ids=list(range(N_CORES)),
                               trace=True, **kw)
    y = np.zeros((B, N, D), dtype=np.float32)
    for c in range(N_CORES):
        y[c // 4] += res.results[c]["y"]
    return y, res
